# revision 1
# baseline (speedup 1.0000x reference)
"""Trainium2 Bass kernel for nn_MultiHeadGraphAttention (N=4096, heads=8, d=64).

Two SPMD launches on 8 NeuronCores:
  L1 (n-sharded): bilinear x = einsum('np,hpq,nq->nh') via the PE diag-trick
     (A^T_q = Xp_chunk.T @ diag(xn[:, q]), fp16 operands, fp32 PSUM accum),
     then xt = x@WtR and s = x@[a-folds] on-device. b_bil folds added on host.
  L2 (head-sharded): core k computes head k's attention for all 4096 queries.
     Layout: keys j on partitions, queries i on free dim. scores built by
     ACT Prelu(a_i-broadcast + b_j bias); per-query max subtracted on DVE;
     exp -> fp16; attn@[xt|1] on PE (fp32 PSUM); normalize by the ones-column
     sum; tanh. Host transposes/concats head outputs.

kernel(**inputs) takes the full unsharded inputs and returns the full output.
"""
import sys
if '/opt/trn_rl_repo' not in sys.path:
    sys.path.insert(0, '/opt/trn_rl_repo')

from contextlib import ExitStack
import numpy as np

import concourse.bacc as bacc
import concourse.tile as tile
from concourse import mybir
from concourse.bass_utils import run_bass_kernel_spmd

f32, f16 = mybir.dt.float32, mybir.dt.float16
AFn = mybir.ActivationFunctionType

N, P, QD, H, K, D = 4096, 128, 128, 256, 8, 64
NLOC = N // 8          # L1 rows per core
NCH = NLOC // 128      # L1 row chunks per core
NSLOT = 8              # A^T ring slots
NJC = N // 128         # L2 key chunks
NBB = 8                # L2 query blocks of 512


def _build_l1(nc, tc, ctx):
    XP_d = nc.dram_tensor("XP16", (NLOC, 128), f16, kind="ExternalInput").ap()
    XN_d = nc.dram_tensor("XN32", (NLOC, 128), f32, kind="ExternalInput").ap()
    WSB_d = nc.dram_tensor("WSB", (128, 128 * 256), f16, kind="ExternalInput").ap()
    ID_d = nc.dram_tensor("IDENT", (128, 128), f16, kind="ExternalInput").ap()
    WTR_d = nc.dram_tensor("WTR", (256, 512), f32, kind="ExternalInput").ap()
    AF_d = nc.dram_tensor("AFM", (256, 16), f32, kind="ExternalInput").ap()
    XTC_d = nc.dram_tensor("XTC", (NLOC, 512), f32, kind="ExternalOutput").ap()
    SC_d = nc.dram_tensor("SC", (NLOC, 16), f32, kind="ExternalOutput").ap()

    const = ctx.enter_context(tc.tile_pool(name="const", bufs=1))
    dpool = ctx.enter_context(tc.tile_pool(name="dpool", bufs=3))
    papool = ctx.enter_context(tc.tile_pool(name="papool", bufs=3, space="PSUM"))
    pxpool = ctx.enter_context(tc.tile_pool(name="pxpool", bufs=1, space="PSUM"))
    opool = ctx.enter_context(tc.tile_pool(name="opool", bufs=1))

    wsb = const.tile([128, 128 * 256], f16, tag="wsb")
    nc.sync.dma_start(wsb[:], WSB_d[:])
    ident = const.tile([128, 128], f16, tag="ident")
    nc.sync.dma_start(ident[:], ID_d[:])
    xpt, xnt = [], []
    for ch in range(NCH):
        xpc = const.tile([128, 128], f16, tag=f"xp{ch}", name=f"xp{ch}")
        nc.sync.dma_start(xpc[:], XP_d[ch * 128:(ch + 1) * 128, :])
        xpt.append(xpc)
        xnc = const.tile([128, 128], f32, tag=f"xn{ch}", name=f"xn{ch}")
        nc.sync.dma_start(xnc[:], XN_d[ch * 128:(ch + 1) * 128, :])
        xnt.append(xnc)
    wtr, afm = [], []
    for hh in range(2):
        wt_h = const.tile([128, 512], f32, tag=f"wtr{hh}", name=f"wtr{hh}")
        nc.sync.dma_start(wt_h[:], WTR_d[hh * 128:(hh + 1) * 128, :])
        wtr.append(wt_h)
        af_h = const.tile([128, 16], f32, tag=f"af{hh}", name=f"af{hh}")
        nc.sync.dma_start(af_h[:], AF_d[hh * 128:(hh + 1) * 128, :])
        afm.append(af_h)

    atbuf = const.tile([128, NSLOT * 512], f16, tag="atbuf")
    atv = atbuf[:].rearrange("p (s n) -> p s n", s=NSLOT)

    pxt = [pxpool.tile([128, 512], f32, tag=f"pxt{hh}", name=f"pxt{hh}")
           for hh in range(2)]

    for g in range(QD // 4):
        for ch in range(NCH):
            dsup = dpool.tile([128, 512], f16, tag="dsup")
            for j in range(4):
                q = 4 * g + j
                nc.vector.tensor_scalar_mul(dsup[:, j * 128:(j + 1) * 128],
                                            ident[:], xnt[ch][:, q:q + 1])
            pa = papool.tile([128, 512], f32, tag="pa")
            nc.tensor.matmul(pa[:], xpt[ch][:], dsup[:], start=True, stop=True)
            s0 = (4 * g) % NSLOT
            dst = atv[:, s0:s0 + 4, ch * 128:(ch + 1) * 128]
            src = pa[:].rearrange("p (j n) -> p j n", j=4)
            if ch % 2 == 0:
                nc.vector.tensor_copy(dst, src)
            else:
                nc.scalar.copy(dst, src)
        for j in range(4):
            q = 4 * g + j
            slot = q % NSLOT
            for hh in range(2):
                nc.tensor.matmul(pxt[hh][:],
                                 wsb[:, q * 256 + hh * 128:q * 256 + hh * 128 + 128],
                                 atv[:, slot, :],
                                 start=(q == 0), stop=(q == QD - 1))

    xts = []
    for hh in range(2):
        xt_h = opool.tile([128, 512], f32, tag=f"xts{hh}", name=f"xts{hh}")
        nc.vector.tensor_copy(xt_h[:], pxt[hh][:])
        xts.append(xt_h)

    with tc.tile_pool(name="p2", bufs=1, space="PSUM") as p2:
        for ch in range(NCH):
            pxt2 = p2.tile([128, 512], f32, tag="pxt2")
            for hh in range(2):
                nc.tensor.matmul(pxt2[:], xts[hh][:, ch * 128:(ch + 1) * 128],
                                 wtr[hh][:], start=(hh == 0), stop=(hh == 1))
            ot = opool.tile([128, 512], f32, tag="ot")
            nc.vector.tensor_copy(ot[:], pxt2[:])
            nc.sync.dma_start(XTC_d[ch * 128:(ch + 1) * 128, :], ot[:])
            ps2 = p2.tile([128, 16], f32, tag="ps2")
            for hh in range(2):
                nc.tensor.matmul(ps2[:], xts[hh][:, ch * 128:(ch + 1) * 128],
                                 afm[hh][:], start=(hh == 0), stop=(hh == 1))
            os_t = opool.tile([128, 16], f32, tag="os")
            nc.scalar.copy(os_t[:], ps2[:])
            nc.sync.dma_start(SC_d[ch * 128:(ch + 1) * 128, :], os_t[:])


def _build_l2(nc, tc, ctx):
    XTSB_d = nc.dram_tensor("XTSB", (128, NJC * 65), f16, kind="ExternalInput").ap()
    SSROW_d = nc.dram_tensor("SSROW", (1, N), f32, kind="ExternalInput").ap()
    MROW_d = nc.dram_tensor("MROW", (1, N), f32, kind="ExternalInput").ap()
    BCOL_d = nc.dram_tensor("BCOL", (128, NJC), f32, kind="ExternalInput").ap()
    ONES1_d = nc.dram_tensor("ONES1", (1, 128), f32, kind="ExternalInput").ap()
    ONES64_d = nc.dram_tensor("ONES64", (1, 64), f32, kind="ExternalInput").ap()
    OUTT_d = nc.dram_tensor("OUTT", (64, N), f32, kind="ExternalOutput").ap()

    const = ctx.enter_context(tc.tile_pool(name="const", bufs=1))
    spool = ctx.enter_context(tc.tile_pool(name="spool", bufs=2))
    opool = ctx.enter_context(tc.tile_pool(name="opool", bufs=1))

    xtsb = const.tile([128, NJC * 65], f16, tag="xtsb")
    nc.sync.dma_start(xtsb[:], XTSB_d[:])
    ssrow = const.tile([1, N], f32, tag="ssrow")
    nc.sync.dma_start(ssrow[:], SSROW_d[:])
    bcol = const.tile([128, NJC], f32, tag="bcol")
    nc.sync.dma_start(bcol[:], BCOL_d[:])
    mrow = const.tile([1, N], f32, tag="mrow")
    nc.sync.dma_start(mrow[:], MROW_d[:])
    ones1 = const.tile([1, 128], f32, tag="ones1")
    nc.sync.dma_start(ones1[:], ONES1_d[:])
    ones64 = const.tile([1, 64], f32, tag="ones64")
    nc.sync.dma_start(ones64[:], ONES64_d[:])

    ssrep = const.tile([128, N], f32, tag="ssrep")
    mrep = const.tile([128, N], f32, tag="mrep")
    with tc.tile_pool(name="pbc", bufs=2, space="PSUM") as pbc:
        for bb in range(NBB):
            pb = pbc.tile([128, 512], f32, tag="pb")
            nc.tensor.matmul(pb[:], ones1[:], ssrow[:, bb * 512:(bb + 1) * 512],
                             start=True, stop=True)
            nc.vector.tensor_copy(ssrep[:, bb * 512:(bb + 1) * 512], pb[:])
            pm = pbc.tile([128, 512], f32, tag="pm")
            nc.tensor.matmul(pm[:], ones1[:], mrow[:, bb * 512:(bb + 1) * 512],
                             start=True, stop=True)
            nc.vector.tensor_copy(mrep[:, bb * 512:(bb + 1) * 512], pm[:])

    with tc.tile_pool(name="pat", bufs=1, space="PSUM") as pat:
        accs = []
        for bb in range(NBB):
            acc = pat.tile([65, 512], f32, tag=f"acc{bb}", name=f"acc{bb}")
            accs.append(acc)
        for jc in range(NJC):
            t = spool.tile([128, N], f32, tag="t")
            nc.scalar.activation(t[:], ssrep[:], AFn.Prelu,
                                 bias=bcol[:, jc:jc + 1], scale=1.0, alpha=0.2)
            nc.vector.tensor_sub(t[:], t[:], mrep[:])
            e = spool.tile([128, N], f16, tag="e")
            nc.scalar.activation(e[:], t[:], AFn.Exp)
            for bb in range(NBB):
                nc.tensor.matmul(accs[bb][:], xtsb[:, jc * 65:(jc + 1) * 65],
                                 e[:, bb * 512:(bb + 1) * 512],
                                 start=(jc == 0), stop=(jc == NJC - 1))
        outu = opool.tile([65, N], f32, tag="outu")
        for bb in range(NBB):
            nc.vector.tensor_copy(outu[:, bb * 512:(bb + 1) * 512], accs[bb][:])

    zinv = opool.tile([1, N], f32, tag="zinv")
    nc.vector.reciprocal(zinv[:], outu[64:65, :])
    ot = opool.tile([64, N], f32, tag="ot")
    with tc.tile_pool(name="pz", bufs=2, space="PSUM") as pz:
        for bb in range(NBB):
            pzt = pz.tile([64, 512], f32, tag="pzt")
            nc.tensor.matmul(pzt[:], ones64[:], zinv[:, bb * 512:(bb + 1) * 512],
                             start=True, stop=True)
            nc.vector.tensor_mul(ot[:, bb * 512:(bb + 1) * 512],
                                 outu[0:64, bb * 512:(bb + 1) * 512], pzt[:])
    nc.scalar.activation(ot[:], ot[:], AFn.Tanh)
    nc.sync.dma_start(OUTT_d[:], ot[:])


_CACHE = {}


def _get_kernels():
    if "l1" not in _CACHE:
        nc1 = bacc.Bacc("TRN2", target_bir_lowering=False, debug=False, num_devices=8)
        with tile.TileContext(nc1) as tc:
            with ExitStack() as ctx:
                _build_l1(nc1, tc, ctx)
        nc1.compile()
        _CACHE["l1"] = nc1
        nc2 = bacc.Bacc("TRN2", target_bir_lowering=False, debug=False, num_devices=8)
        with tile.TileContext(nc2) as tc:
            with ExitStack() as ctx:
                _build_l2(nc2, tc, ctx)
        nc2.compile()
        _CACHE["l2"] = nc2
    return _CACHE["l1"], _CACHE["l2"]


def kernel(x_prices, x_news, W_bil, b_bil, Wt, a_vec):
    xp = np.asarray(x_prices, np.float32)
    xn = np.asarray(x_news, np.float32)
    W = np.asarray(W_bil, np.float32)
    bb_ = np.asarray(b_bil, np.float32)
    Wt_ = np.asarray(Wt, np.float32)
    av = np.asarray(a_vec, np.float32)

    nc1, nc2 = _get_kernels()

    # ---- L1 host prep ----
    WSB = np.ascontiguousarray(W.transpose(1, 2, 0).reshape(128, 128 * 256)).astype(np.float16)
    WTR = np.ascontiguousarray(Wt_.transpose(2, 0, 1).reshape(256, 512)).astype(np.float32)
    AFM = np.concatenate([(Wt_ * av[:, None, :D].transpose(0, 2, 1)).sum(1).T,
                          (Wt_ * av[:, None, D:].transpose(0, 2, 1)).sum(1).T], axis=1)
    AFM = np.ascontiguousarray(AFM).astype(np.float32)
    IDENT = np.eye(128, dtype=np.float16)
    in1 = []
    for c in range(8):
        sl = slice(c * NLOC, (c + 1) * NLOC)
        in1.append({"XP16": xp[sl].astype(np.float16),
                    "XN32": xn[sl],
                    "WSB": WSB, "IDENT": IDENT, "WTR": WTR, "AFM": AFM})
    r1 = run_bass_kernel_spmd(nc1, in1, core_ids=list(range(8)))

    # ---- host glue: gather, add b_bil folds, build per-head L2 inputs ----
    xt_dev = np.concatenate([r1.results[c]["XTC"] for c in range(8)], 0)
    s_dev = np.concatenate([r1.results[c]["SC"] for c in range(8)], 0)
    xt_full = xt_dev + (bb_ @ WTR)                       # (N, 512)
    s_full = s_dev + (bb_ @ AFM)                         # (N, 16)
    xt_hd = xt_full.reshape(N, K, D)
    ss = s_full[:, :8].T                                 # (8, N)
    sd = s_full[:, 8:].T

    in2 = []
    ones1 = np.ones((1, 128), np.float32)
    ones64 = np.ones((1, 64), np.float32)
    for k in range(K):
        xt1k = np.concatenate([xt_hd[:, k, :], np.ones((N, 1), np.float32)], 1)
        xtsb = np.ascontiguousarray(
            xt1k.reshape(NJC, 128, 65).transpose(1, 0, 2).reshape(128, NJC * 65)
        ).astype(np.float16)
        mxr = ss[k] + sd[k].max()
        m = np.where(mxr >= 0, mxr, np.float32(0.2) * mxr).astype(np.float32)
        in2.append({"XTSB": xtsb,
                    "SSROW": np.ascontiguousarray(ss[k][None, :]),
                    "MROW": m[None, :],
                    "BCOL": np.ascontiguousarray(sd[k].reshape(NJC, 128).T),
                    "ONES1": ones1, "ONES64": ones64})
    r2 = run_bass_kernel_spmd(nc2, in2, core_ids=list(range(8)))

    out = np.empty((N, K * D), np.float32)
    for k in range(K):
        out[:, k * D:(k + 1) * D] = r2.results[k]["OUTT"].T
    return out


# revision 5
# speedup vs baseline: 5377.2345x; 5377.2345x over previous
"""Trainium2 Bass kernel for nn_MultiHeadGraphAttention (N=4096, heads=8, d=64).

Two SPMD launches on 8 NeuronCores:
  L1 (n-sharded): bilinear x = einsum('np,hpq,nq->nh') via the PE diag-trick
     (A^T_q = Xp_chunk.T @ diag(xn[:, q]), fp16 operands, fp32 PSUM accum),
     then xt = x@WtR and s = x@[a-folds] on-device. b_bil folds added on host.
  L2 (head-sharded): core k computes head k's attention for all 4096 queries.
     Layout: keys j on partitions, queries i on free dim. scores built by
     ACT Prelu(a_i-broadcast + b_j bias); per-query max subtracted on DVE;
     exp -> fp16; attn@[xt|1] on PE (fp32 PSUM); normalize by the ones-column
     sum; tanh. Host transposes/concats head outputs.

kernel(**inputs) takes the full unsharded inputs and returns the full output.
"""
import sys
if '/opt/trn_rl_repo' not in sys.path:
    sys.path.insert(0, '/opt/trn_rl_repo')

from contextlib import ExitStack
import numpy as np

import concourse.bacc as bacc
import concourse.tile as tile
from concourse import mybir
from concourse.bass_utils import run_bass_kernel_spmd

f32, f16 = mybir.dt.float32, mybir.dt.float16
AFn = mybir.ActivationFunctionType

N, P, QD, H, K, D = 4096, 128, 128, 256, 8, 64
NLOC = N // 8          # L1 rows per core
NCH = NLOC // 128      # L1 row chunks per core
NSLOT = 8              # A^T ring slots
NJC = N // 128         # L2 key chunks
NBB = 8                # L2 query blocks of 512


def _build_l1(nc, tc, ctx):
    XP_d = nc.dram_tensor("XP16", (NLOC, 128), f16, kind="ExternalInput").ap()
    XN_d = nc.dram_tensor("XN32", (NLOC, 128), f32, kind="ExternalInput").ap()
    WSB_d = nc.dram_tensor("WSB", (128, 128 * 256), f16, kind="ExternalInput").ap()
    ID_d = nc.dram_tensor("IDENT", (128, 128), f16, kind="ExternalInput").ap()
    WTR_d = nc.dram_tensor("WTR", (256, 512), f32, kind="ExternalInput").ap()
    AF_d = nc.dram_tensor("AFM", (256, 16), f32, kind="ExternalInput").ap()
    XTC_d = nc.dram_tensor("XTC", (NLOC, 512), f32, kind="ExternalOutput").ap()
    SC_d = nc.dram_tensor("SC", (NLOC, 16), f32, kind="ExternalOutput").ap()

    const = ctx.enter_context(tc.tile_pool(name="const", bufs=1))
    dpool = ctx.enter_context(tc.tile_pool(name="dpool", bufs=3))
    papool = ctx.enter_context(tc.tile_pool(name="papool", bufs=3, space="PSUM"))
    pxpool = ctx.enter_context(tc.tile_pool(name="pxpool", bufs=1, space="PSUM"))
    opool = ctx.enter_context(tc.tile_pool(name="opool", bufs=1))

    wsb = const.tile([128, 128 * 256], f16, tag="wsb")
    nc.sync.dma_start(wsb[:], WSB_d[:])
    ident = const.tile([128, 128], f16, tag="ident")
    nc.sync.dma_start(ident[:], ID_d[:])
    xpt, xnt = [], []
    for ch in range(NCH):
        xpc = const.tile([128, 128], f16, tag=f"xp{ch}", name=f"xp{ch}")
        nc.sync.dma_start(xpc[:], XP_d[ch * 128:(ch + 1) * 128, :])
        xpt.append(xpc)
        xnc = const.tile([128, 128], f32, tag=f"xn{ch}", name=f"xn{ch}")
        nc.sync.dma_start(xnc[:], XN_d[ch * 128:(ch + 1) * 128, :])
        xnt.append(xnc)
    wtr, afm = [], []
    for hh in range(2):
        wt_h = const.tile([128, 512], f32, tag=f"wtr{hh}", name=f"wtr{hh}")
        nc.sync.dma_start(wt_h[:], WTR_d[hh * 128:(hh + 1) * 128, :])
        wtr.append(wt_h)
        af_h = const.tile([128, 16], f32, tag=f"af{hh}", name=f"af{hh}")
        nc.sync.dma_start(af_h[:], AF_d[hh * 128:(hh + 1) * 128, :])
        afm.append(af_h)

    atbuf = const.tile([128, NSLOT * 512], f16, tag="atbuf")
    atv = atbuf[:].rearrange("p (s n) -> p s n", s=NSLOT)

    pxt = [pxpool.tile([128, 512], f32, tag=f"pxt{hh}", name=f"pxt{hh}")
           for hh in range(2)]

    for g in range(QD // 4):
        for ch in range(NCH):
            dsup = dpool.tile([128, 512], f16, tag="dsup")
            for j in range(4):
                q = 4 * g + j
                nc.vector.tensor_scalar_mul(dsup[:, j * 128:(j + 1) * 128],
                                            ident[:], xnt[ch][:, q:q + 1])
            pa = papool.tile([128, 512], f32, tag="pa")
            nc.tensor.matmul(pa[:], xpt[ch][:], dsup[:], start=True, stop=True)
            s0 = (4 * g) % NSLOT
            dst = atv[:, s0:s0 + 4, ch * 128:(ch + 1) * 128]
            src = pa[:].rearrange("p (j n) -> p j n", j=4)
            if ch % 2 == 0:
                nc.vector.tensor_copy(dst, src)
            else:
                nc.scalar.copy(dst, src)
        for j in range(4):
            q = 4 * g + j
            slot = q % NSLOT
            for hh in range(2):
                nc.tensor.matmul(pxt[hh][:],
                                 wsb[:, q * 256 + hh * 128:q * 256 + hh * 128 + 128],
                                 atv[:, slot, :],
                                 start=(q == 0), stop=(q == QD - 1))

    xts = []
    for hh in range(2):
        xt_h = opool.tile([128, 512], f32, tag=f"xts{hh}", name=f"xts{hh}")
        nc.vector.tensor_copy(xt_h[:], pxt[hh][:])
        xts.append(xt_h)

    with tc.tile_pool(name="p2", bufs=1, space="PSUM") as p2:
        for ch in range(NCH):
            pxt2 = p2.tile([128, 512], f32, tag="pxt2")
            for hh in range(2):
                nc.tensor.matmul(pxt2[:], xts[hh][:, ch * 128:(ch + 1) * 128],
                                 wtr[hh][:], start=(hh == 0), stop=(hh == 1))
            ot = opool.tile([128, 512], f32, tag="ot")
            nc.vector.tensor_copy(ot[:], pxt2[:])
            nc.sync.dma_start(XTC_d[ch * 128:(ch + 1) * 128, :], ot[:])
            ps2 = p2.tile([128, 16], f32, tag="ps2")
            for hh in range(2):
                nc.tensor.matmul(ps2[:], xts[hh][:, ch * 128:(ch + 1) * 128],
                                 afm[hh][:], start=(hh == 0), stop=(hh == 1))
            os_t = opool.tile([128, 16], f32, tag="os")
            nc.scalar.copy(os_t[:], ps2[:])
            nc.sync.dma_start(SC_d[ch * 128:(ch + 1) * 128, :], os_t[:])


def _build_l2(nc, tc, ctx):
    """Factored-exponential attention for one head:
      e[j,i] = exp(leaky(a_i+b_j) - m_i) = max(v_j*u1_i, vh_j*u2_i)
    with u1 = exp(a+bmax-m), u2 = exp(0.2a+0.2bmax-m), v = exp(b-bmax),
    vh = exp(0.2(b-bmax)) — all factors in (0, 1], fp16-safe.
    Exp ARGS are shipped; the tiny exps run on device."""
    XTSB_d = nc.dram_tensor("XTSB", (128, NJC * 65), f16, kind="ExternalInput").ap()
    U1ARG_d = nc.dram_tensor("U1ARG", (1, N), f32, kind="ExternalInput").ap()
    U2ARG_d = nc.dram_tensor("U2ARG", (1, N), f32, kind="ExternalInput").ap()
    VARG_d = nc.dram_tensor("VARG", (128, NJC), f32, kind="ExternalInput").ap()
    V2ARG_d = nc.dram_tensor("V2ARG", (128, NJC), f32, kind="ExternalInput").ap()
    ONES1_d = nc.dram_tensor("ONES1", (1, 128), f32, kind="ExternalInput").ap()
    ONES64_d = nc.dram_tensor("ONES64", (1, 64), f32, kind="ExternalInput").ap()
    OUTT_d = nc.dram_tensor("OUTT", (64, N), f32, kind="ExternalOutput").ap()

    const = ctx.enter_context(tc.tile_pool(name="const", bufs=1))
    spool = ctx.enter_context(tc.tile_pool(name="spool", bufs=3))
    opool = ctx.enter_context(tc.tile_pool(name="opool", bufs=1))

    xtsb = const.tile([128, NJC * 65], f16, tag="xtsb")
    nc.sync.dma_start(xtsb[:], XTSB_d[:])
    u1arg = const.tile([1, N], f32, tag="u1arg")
    nc.sync.dma_start(u1arg[:], U1ARG_d[:])
    u2arg = const.tile([1, N], f32, tag="u2arg")
    nc.sync.dma_start(u2arg[:], U2ARG_d[:])
    varg = const.tile([128, NJC], f32, tag="varg")
    nc.sync.dma_start(varg[:], VARG_d[:])
    v2arg = const.tile([128, NJC], f32, tag="v2arg")
    nc.sync.dma_start(v2arg[:], V2ARG_d[:])
    ones1 = const.tile([1, 128], f32, tag="ones1")
    nc.sync.dma_start(ones1[:], ONES1_d[:])
    ones64 = const.tile([1, 64], f32, tag="ones64")
    nc.sync.dma_start(ones64[:], ONES64_d[:])

    # tiny exps on device
    u1row = const.tile([1, N], f32, tag="u1row")
    nc.scalar.activation(u1row[:], u1arg[:], AFn.Exp)
    u2row = const.tile([1, N], f32, tag="u2row")
    nc.scalar.activation(u2row[:], u2arg[:], AFn.Exp)
    vcol = const.tile([128, NJC], f32, tag="vcol")
    nc.scalar.activation(vcol[:], varg[:], AFn.Exp)
    v2col = const.tile([128, NJC], f32, tag="v2col")
    nc.scalar.activation(v2col[:], v2arg[:], AFn.Exp)

    # broadcast u1/u2 across partitions (PE ones-matmul), store fp16
    u1rep = const.tile([128, N], f16, tag="u1rep")
    u2rep = const.tile([128, N], f16, tag="u2rep")
    with tc.tile_pool(name="pbc", bufs=2, space="PSUM") as pbc:
        for bb in range(NBB):
            pb = pbc.tile([128, 512], f32, tag="pb")
            nc.tensor.matmul(pb[:], ones1[:], u1row[:, bb * 512:(bb + 1) * 512],
                             start=True, stop=True)
            nc.vector.tensor_copy(u1rep[:, bb * 512:(bb + 1) * 512], pb[:])
            pm = pbc.tile([128, 512], f32, tag="pm")
            nc.tensor.matmul(pm[:], ones1[:], u2row[:, bb * 512:(bb + 1) * 512],
                             start=True, stop=True)
            nc.vector.tensor_copy(u2rep[:, bb * 512:(bb + 1) * 512], pm[:])

    with tc.tile_pool(name="pat", bufs=1, space="PSUM") as pat:
        accs = []
        for bb in range(NBB):
            acc = pat.tile([65, 512], f32, tag=f"acc{bb}", name=f"acc{bb}")
            accs.append(acc)
        for jc in range(NJC):
            t1 = spool.tile([128, N], f16, tag="t1")
            nc.vector.tensor_scalar_mul(t1[:], u1rep[:], vcol[:, jc:jc + 1])
            t2 = spool.tile([128, N], f16, tag="t2")
            nc.scalar.activation(t2[:], u2rep[:], AFn.Copy,
                                 scale=v2col[:, jc:jc + 1])
            e = spool.tile([128, N], f16, tag="e")
            nc.vector.tensor_max(e[:], t1[:], t2[:])
            for bb in range(NBB):
                nc.tensor.matmul(accs[bb][:], xtsb[:, jc * 65:(jc + 1) * 65],
                                 e[:, bb * 512:(bb + 1) * 512],
                                 start=(jc == 0), stop=(jc == NJC - 1))
        outu = opool.tile([65, N], f32, tag="outu")
        for bb in range(NBB):
            nc.vector.tensor_copy(outu[:, bb * 512:(bb + 1) * 512], accs[bb][:])

    zinv = opool.tile([1, N], f32, tag="zinv")
    nc.vector.reciprocal(zinv[:], outu[64:65, :])
    ot = opool.tile([64, N], f32, tag="ot")
    with tc.tile_pool(name="pz", bufs=2, space="PSUM") as pz:
        for bb in range(NBB):
            pzt = pz.tile([64, 512], f32, tag="pzt")
            nc.tensor.matmul(pzt[:], ones64[:], zinv[:, bb * 512:(bb + 1) * 512],
                             start=True, stop=True)
            nc.vector.tensor_mul(ot[:, bb * 512:(bb + 1) * 512],
                                 outu[0:64, bb * 512:(bb + 1) * 512], pzt[:])
    nc.scalar.activation(ot[:], ot[:], AFn.Tanh)
    nc.sync.dma_start(OUTT_d[:], ot[:])


_CACHE = {}


def _run_spmd(nc, in_maps):
    """run_bass_kernel_spmd with one retry for transient device errors."""
    try:
        return run_bass_kernel_spmd(nc, in_maps, core_ids=list(range(8)))
    except Exception:
        return run_bass_kernel_spmd(nc, in_maps, core_ids=list(range(8)))


def _get_kernels():
    if "l1" not in _CACHE:
        nc1 = bacc.Bacc("TRN2", target_bir_lowering=False, debug=False, num_devices=8)
        with tile.TileContext(nc1) as tc:
            with ExitStack() as ctx:
                _build_l1(nc1, tc, ctx)
        nc1.compile()
        _CACHE["l1"] = nc1
        nc2 = bacc.Bacc("TRN2", target_bir_lowering=False, debug=False, num_devices=8)
        with tile.TileContext(nc2) as tc:
            with ExitStack() as ctx:
                _build_l2(nc2, tc, ctx)
        nc2.compile()
        _CACHE["l2"] = nc2
    return _CACHE["l1"], _CACHE["l2"]


def kernel(x_prices, x_news, W_bil, b_bil, Wt, a_vec):
    xp = np.asarray(x_prices, np.float32)
    xn = np.asarray(x_news, np.float32)
    W = np.asarray(W_bil, np.float32)
    bb_ = np.asarray(b_bil, np.float32)
    Wt_ = np.asarray(Wt, np.float32)
    av = np.asarray(a_vec, np.float32)

    nc1, nc2 = _get_kernels()

    # ---- L1 host prep ----
    WSB = np.ascontiguousarray(W.transpose(1, 2, 0).reshape(128, 128 * 256)).astype(np.float16)
    WTR = np.ascontiguousarray(Wt_.transpose(2, 0, 1).reshape(256, 512)).astype(np.float32)
    AFM = np.concatenate([(Wt_ * av[:, None, :D].transpose(0, 2, 1)).sum(1).T,
                          (Wt_ * av[:, None, D:].transpose(0, 2, 1)).sum(1).T], axis=1)
    AFM = np.ascontiguousarray(AFM).astype(np.float32)
    IDENT = np.eye(128, dtype=np.float16)
    in1 = []
    for c in range(8):
        sl = slice(c * NLOC, (c + 1) * NLOC)
        in1.append({"XP16": xp[sl].astype(np.float16),
                    "XN32": xn[sl],
                    "WSB": WSB, "IDENT": IDENT, "WTR": WTR, "AFM": AFM})
    r1 = _run_spmd(nc1, in1)

    # ---- host glue: gather, add b_bil folds, build per-head L2 inputs ----
    xt_dev = np.concatenate([r1.results[c]["XTC"] for c in range(8)], 0)
    s_dev = np.concatenate([r1.results[c]["SC"] for c in range(8)], 0)
    xt_full = xt_dev + (bb_ @ WTR)                       # (N, 512)
    s_full = s_dev + (bb_ @ AFM)                         # (N, 16)
    xt_hd = xt_full.reshape(N, K, D)
    ss = s_full[:, :8].T                                 # (8, N)
    sd = s_full[:, 8:].T

    in2 = []
    ones1 = np.ones((1, 128), np.float32)
    ones64 = np.ones((1, 64), np.float32)
    for k in range(K):
        xt1k = np.concatenate([xt_hd[:, k, :], np.ones((N, 1), np.float32)], 1)
        xtsb = np.ascontiguousarray(
            xt1k.reshape(NJC, 128, 65).transpose(1, 0, 2).reshape(128, NJC * 65)
        ).astype(np.float16)
        bmax = sd[k].max()
        mxr = ss[k] + bmax
        m = np.where(mxr >= 0, mxr, np.float32(0.2) * mxr).astype(np.float32)
        u1a = (ss[k] + bmax - m).astype(np.float32)          # in (-inf, 0]
        u2a = (np.float32(0.2) * (ss[k] + bmax) - m).astype(np.float32)
        va = (sd[k] - bmax).astype(np.float32)
        v2a = (np.float32(0.2) * (sd[k] - bmax)).astype(np.float32)
        in2.append({"XTSB": xtsb,
                    "U1ARG": np.ascontiguousarray(u1a[None, :]),
                    "U2ARG": np.ascontiguousarray(u2a[None, :]),
                    "VARG": np.ascontiguousarray(va.reshape(NJC, 128).T),
                    "V2ARG": np.ascontiguousarray(v2a.reshape(NJC, 128).T),
                    "ONES1": ones1, "ONES64": ones64})
    r2 = _run_spmd(nc2, in2)

    out = np.empty((N, K * D), np.float32)
    for k in range(K):
        out[:, k * D:(k + 1) * D] = r2.results[k]["OUTT"].T
    return out
